# revision 5
# baseline (speedup 1.0000x reference)
"""Trainium2 Bass kernel for a 2-layer bipartite GraphSAGE encoder
(disease<->gene, 25000/20000 nodes, 1M edges, all feature dims 128).

Strategy (8 NeuronCores, SPMD):
  - Shard aggregation *destinations*: core k owns genes [2500k, 2500k+2500)
    and diseases [3125k, 3125k+3125). Host sorts edges by dst (for gene
    aggregation) and by src (for disease aggregation); each core processes
    exactly the edges that target its node range, so no all-reduce of
    segment sums is needed -- only an AllGather of layer-1 node features.
  - Segment-mean on device: per 128-edge tile, dma_gather the source rows
    (bf16, edges on partitions), build onehot[e,b] = (iota==lo_e) * r_e on
    the vector engine (r_e = 1/max(deg,1) folds the mean divide), and
    accumulate meanT[f,b] += msg[e,f]^T @ onehot on the tensor engine (PSUM,
    one 128-bucket window per accumulation group).
  - Dense stages run in transposed layout: hT = W_l^T meanT + W_r^T xT + b,
    f32. Layer-1 shard results are PE-transposed to row-major bf16 tables
    and AllGathered for layer-2 gathers; final outputs are PE-transposed to
    row-major f32 shards and concatenated on host.
"""
import sys

if '/opt/trn_rl_repo' not in sys.path:
    sys.path.insert(0, '/opt/trn_rl_repo')

import numpy as np

import concourse.bacc as bacc
import concourse.mybir as mybir
import concourse.tile as tile
from concourse.bass_utils import run_bass_kernel_spmd
from concourse.vector_clock import ScopedClock, VectorClock

F32 = mybir.dt.float32
BF16 = mybir.dt.bfloat16
I16 = mybir.dt.int16
NP_BF16 = mybir.dt.np(BF16)

ND, NG, E = 25000, 20000, 1_000_000
F = 128
NCORES = 8


class _TileContextSplitDrain(tile.TileContext):
    """Work around a walrus limit on sync-wait commands per Drain: emit one
    drain per (proc, tick) of the global clock, each with <=1 sem wait."""

    def _drain_and_barrier(self, tick_clock, wait_clock):
        gvc = tick_clock.global_clock
        n = len(gvc)
        for proc in range(n):
            t = gvc[proc]
            if t > 0:
                vec = [0] * n
                vec[proc] = t
                d = self.nc.sync.drain()
                wait_clock.add_sem_waits(d.ins, ScopedClock({None: VectorClock(vec)}))
        self.nc.all_engine_barrier()
        assert self.sems is not None
        popped = self.nc._tile_sem_poison_stack.pop()
        assert popped is self._sem_poison
        self.nc.clear_and_free_semaphores(list(self.sems.allocated().values()))
        self.nc.all_engine_barrier()


def _pass_meta(sortk, other, r_node, n_own, nw, ncores):
    """Per-core gather/one-hot metadata for one aggregation direction.

    sortk: [E] destination node id of each edge (aggregation key)
    other: [E] node id each edge gathers its message from
    r_node: [Ndst] f32 1/max(degree,1)
    n_own:  destination nodes per core (contiguous ranges, in core order)
    nw:     windows (of 128 destination buckets) per core

    Returns (idx16 [C,128,8*TOT] i16, lo [C,128,TOT] f32, rv [C,128,TOT] f32,
             tw list[nw], TOT).
    """
    order = np.argsort(sortk, kind="stable")
    ks = np.asarray(sortk)[order].astype(np.int64)
    os_ = np.asarray(other)[order].astype(np.int64)
    bounds = np.searchsorted(ks, np.arange(ncores + 1) * n_own)

    percore = []
    cnt_mat = np.zeros((ncores, nw), np.int64)
    for k in range(ncores):
        s, e = int(bounds[k]), int(bounds[k + 1])
        loc = ks[s:e] - k * n_own
        w = loc >> 7
        cnt_mat[k] = np.bincount(w, minlength=nw)
        percore.append((s, e, loc, w))
    tw = np.maximum(1, -(-cnt_mat // 128)).max(axis=0)          # [nw]
    To = np.concatenate([[0], np.cumsum(tw)]).astype(np.int64)  # [nw+1]
    tot = int(To[-1])

    idx16 = np.zeros((ncores, 16, 8 * tot), np.int16)
    lo = np.full((ncores, 128, tot), 200.0, np.float32)
    rv = np.zeros((ncores, 128, tot), np.float32)
    for k in range(ncores):
        s, e, loc, w = percore[k]
        n = e - s
        if n == 0:
            continue
        offs = np.concatenate([[0], np.cumsum(cnt_mat[k])])[:-1]
        j = np.arange(n) - offs[w]          # slot within window
        tcol = To[w] + (j >> 7)             # tile column in lo/rv
        lane = j & 127
        lo[k, lane, tcol] = (loc - (w << 7)).astype(np.float32)
        rv[k, lane, tcol] = r_node[ks[s:e]]
        icol = 8 * To[w] + (j >> 4)
        idx16[k, j & 15, icol] = os_[s:e].astype(np.int16)
    idx_full = np.tile(idx16, (1, 8, 1))    # replicate across 8x16 partitions
    return idx_full, lo, rv, [int(x) for x in tw], tot


def _build_program(nd, ng, twg, twd, totg, totd):
    d_sh, g_sh = nd // NCORES, ng // NCORES
    gw, dw = len(twg), len(twd)
    g_pad, d_pad = gw * 128, dw * 128

    nc = bacc.Bacc("TRN2", num_devices=NCORES)

    def inp(name, shape, dt):
        return nc.dram_tensor(name, shape, dt, kind="ExternalInput")

    xd_bf = inp("xd_bf", [nd, F], BF16)
    xg_bf = inp("xg_bf", [ng, F], BF16)
    xgT = inp("xgT", [F, g_pad], F32)
    xdT = inp("xdT", [F, d_pad], F32)
    gidx = inp("gidx", [128, 8 * totg], I16)
    glo = inp("glo", [128, totg], F32)
    gr = inp("gr", [128, totg], F32)
    didx = inp("didx", [128, 8 * totd], I16)
    dlo = inp("dlo", [128, totd], F32)
    dr = inp("dr", [128, totd], F32)
    wts = {w: inp(w, [F, F], F32)
           for w in ("w1dgl", "w1dgr", "w1gdl", "w1gdr",
                     "w2dgl", "w2dgr", "w2gdl", "w2gdr")}
    bss = {b: inp(b, [F, 1], F32) for b in ("b1dg", "b1gd", "b2dg", "b2gd")}
    iota_in = inp("iota", [128, 128], BF16)
    ident_in = inp("ident", [128, 128], F32)
    g2o = nc.dram_tensor("g2o", [g_sh, F], F32, kind="ExternalOutput")
    d2o = nc.dram_tensor("d2o", [d_sh, F], F32, kind="ExternalOutput")

    with _TileContextSplitDrain(nc) as tc:
        with (
            tc.tile_pool(name="const", bufs=1) as cpool,
            tc.tile_pool(name="meanT", bufs=2) as mpool,
            tc.tile_pool(name="stage", bufs=2) as spool,
            tc.tile_pool(name="oh", bufs=4) as ohpool,
            tc.tile_pool(name="tmp", bufs=3) as tpool,
            tc.tile_pool(name="agg_ps", bufs=3, space="PSUM") as agg_ps,
            tc.tile_pool(name="dense_ps", bufs=2, space="PSUM") as dense_ps,
            tc.tile_pool(name="tp_ps", bufs=2, space="PSUM") as tp_ps,
            tc.tile_pool(name="dram", bufs=1, space="DRAM") as dram,
        ):
            def load_const(src, shape, dt):
                t = cpool.tile(shape, dt, tag=src.name)
                nc.sync.dma_start(t[:], src[:])
                return t

            iota_sb = load_const(iota_in, [128, 128], BF16)
            ident_sb = load_const(ident_in, [128, 128], F32)
            w_sb = {k: load_const(v, [F, F], F32) for k, v in wts.items()}
            b_sb = {k: load_const(v, [F, 1], F32) for k, v in bss.items()}
            xgT_sb = load_const(xgT, [F, g_pad], F32)
            xdT_sb = load_const(xdT, [F, d_pad], F32)
            gidx_sb = load_const(gidx, [128, 8 * totg], I16)
            glo_sb = load_const(glo, [128, totg], F32)
            gr_sb = load_const(gr, [128, totg], F32)
            didx_sb = load_const(didx, [128, 8 * totd], I16)
            dlo_sb = load_const(dlo, [128, totd], F32)
            dr_sb = load_const(dr, [128, totd], F32)

            h1gT = cpool.tile([128, g_pad], F32, tag="h1gT")
            h1dT = cpool.tile([128, d_pad], F32, tag="h1dT")

            h1g_rows = dram.tile([g_sh, F], BF16)
            h1d_rows = dram.tile([d_sh, F], BF16)
            h1g_full = dram.tile([ng, F], BF16)
            h1d_full = dram.tile([nd, F], BF16)

            def agg(table_ap, idx_sb, lo_sb, r_sb, tw, meanT, tag):
                to = 0
                for w, tww in enumerate(tw):
                    stage = spool.tile([128, tww * F], BF16, tag="stage")
                    nc.gpsimd.dma_gather(
                        out_ap=stage[:].rearrange("p (t f) -> p t f", f=F),
                        in_ap=table_ap,
                        idxs_ap=idx_sb[:, 8 * to:8 * (to + tww)],
                        num_idxs=tww * 128,
                        num_idxs_reg=tww * 128,
                        elem_size=F,
                        single_packet=False,
                    )
                    ps = agg_ps.tile([128, 128], F32, tag="agg")
                    for t in range(tww):
                        oh = ohpool.tile([128, 128], BF16, tag="oh")
                        nc.vector.tensor_scalar(
                            oh[:], iota_sb[:],
                            lo_sb[:, to + t:to + t + 1],
                            r_sb[:, to + t:to + t + 1],
                            mybir.AluOpType.is_equal, mybir.AluOpType.mult,
                        )
                        nc.tensor.matmul(
                            ps[:], lhsT=stage[:, t * F:(t + 1) * F], rhs=oh[:],
                            start=(t == 0), stop=(t == tww - 1),
                        )
                    nc.vector.tensor_copy(meanT[:, w * 128:(w + 1) * 128], ps[:])
                    to += tww

            def dense(meanT, selfT, wl, wr, bias, outT, npad):
                for c0 in range(0, npad, 512):
                    n = min(512, npad - c0)
                    ps = dense_ps.tile([128, 512], F32, tag="dense")
                    nc.tensor.matmul(ps[:, :n], lhsT=wl[:], rhs=meanT[:, c0:c0 + n],
                                     start=True, stop=False)
                    nc.tensor.matmul(ps[:, :n], lhsT=wr[:], rhs=selfT[:, c0:c0 + n],
                                     start=False, stop=True)
                    nc.vector.tensor_scalar(
                        outT[:, c0:c0 + n], ps[:, :n], bias[:], None,
                        mybir.AluOpType.add,
                    )

            def emit_rows(srcT, n_real, rows_dram, dt):
                for b in range(-(-n_real // 128)):
                    rows = min(128, n_real - b * 128)
                    tp = tp_ps.tile([128, 128], F32, tag="tp")
                    nc.tensor.transpose(tp[:], srcT[:, b * 128:(b + 1) * 128],
                                        ident_sb[:])
                    sb = tpool.tile([128, 128], dt, tag=f"emit_{dt}")
                    nc.vector.tensor_copy(sb[:], tp[:])
                    nc.sync.dma_start(rows_dram[b * 128:b * 128 + rows, :],
                                      sb[:rows, :])

            import os
            stage_lvl = int(os.environ.get("K_STAGE", "5"))

            # ---- layer 1 ----
            meanTg = mpool.tile([128, g_pad], F32, tag="meanT")
            agg(xd_bf[:], gidx_sb, glo_sb, gr_sb, twg, meanTg, "g1")
            dense(meanTg, xgT_sb, w_sb["w1dgl"], w_sb["w1dgr"], b_sb["b1dg"],
                  h1gT, g_pad)
            if stage_lvl >= 2:
                meanTd = mpool.tile([128, d_pad], F32, tag="meanT")
                agg(xg_bf[:], didx_sb, dlo_sb, dr_sb, twd, meanTd, "d1")
                dense(meanTd, xdT_sb, w_sb["w1gdl"], w_sb["w1gdr"], b_sb["b1gd"],
                      h1dT, d_pad)

            emit_rows(h1gT, g_sh, h1g_rows, BF16)
            if stage_lvl >= 2:
                emit_rows(h1dT, d_sh, h1d_rows, BF16)
            if stage_lvl >= 3:
                nc.gpsimd.collective_compute(
                    "AllGather", mybir.AluOpType.bypass,
                    replica_groups=[list(range(NCORES))],
                    ins=[h1g_rows[:]], outs=[h1g_full[:]],
                )
                nc.gpsimd.collective_compute(
                    "AllGather", mybir.AluOpType.bypass,
                    replica_groups=[list(range(NCORES))],
                    ins=[h1d_rows[:]], outs=[h1d_full[:]],
                )

            # ---- layer 2 ----
            if stage_lvl >= 4:
                meanTg2 = mpool.tile([128, g_pad], F32, tag="meanT")
                agg(h1d_full[:], gidx_sb, glo_sb, gr_sb, twg, meanTg2, "g2")
                g2T = mpool.tile([128, g_pad], F32, tag="outT")
                dense(meanTg2, h1gT, w_sb["w2dgl"], w_sb["w2dgr"], b_sb["b2dg"],
                      g2T, g_pad)
            else:
                g2T = h1gT
            emit_rows(g2T, g_sh, g2o[:], F32)

            if stage_lvl >= 5:
                meanTd2 = mpool.tile([128, d_pad], F32, tag="meanT")
                agg(h1g_full[:], didx_sb, dlo_sb, dr_sb, twd, meanTd2, "d2")
                d2T = mpool.tile([128, d_pad], F32, tag="outT")
                dense(meanTd2, h1dT, w_sb["w2gdl"], w_sb["w2gdr"], b_sb["b2gd"],
                      d2T, d_pad)
                emit_rows(d2T, d_sh, d2o[:], F32)
            elif stage_lvl >= 2:
                emit_rows(h1dT, d_sh, d2o[:], F32)

    nc.compile()
    return nc


def _prepare(inputs, nd, ng):
    """Host-side sharding + metadata. Returns (in_maps, static) where static
    carries the per-pass tile counts baked into the program."""
    d_sh, g_sh = nd // NCORES, ng // NCORES
    gw = -(-g_sh // 128)
    dw = -(-d_sh // 128)
    g_pad, d_pad = gw * 128, dw * 128

    x_d = np.asarray(inputs["x_disease"], np.float32)
    x_g = np.asarray(inputs["x_gene"], np.float32)
    src = np.asarray(inputs["src"], np.int64)
    dst = np.asarray(inputs["dst"], np.int64)

    cnt_g = np.bincount(dst, minlength=ng).astype(np.float32)
    cnt_d = np.bincount(src, minlength=nd).astype(np.float32)
    r_g = (1.0 / np.maximum(cnt_g, 1.0)).astype(np.float32)
    r_d = (1.0 / np.maximum(cnt_d, 1.0)).astype(np.float32)

    gidx, glo, gr, twg, totg = _pass_meta(dst, src, r_g, g_sh, gw, NCORES)
    didx, dlo, dr, twd, totd = _pass_meta(src, dst, r_d, d_sh, dw, NCORES)

    iota_np = np.tile(np.arange(128, dtype=np.float64), (128, 1)).astype(NP_BF16)
    ident_np = np.eye(128, dtype=np.float32)
    xd_bf = x_d.astype(NP_BF16)
    xg_bf = x_g.astype(NP_BF16)

    in_maps = []
    for k in range(NCORES):
        xgT = np.zeros((F, g_pad), np.float32)
        xgT[:, :g_sh] = x_g[k * g_sh:(k + 1) * g_sh].T
        xdT = np.zeros((F, d_pad), np.float32)
        xdT[:, :d_sh] = x_d[k * d_sh:(k + 1) * d_sh].T
        m = {
            "xd_bf": xd_bf, "xg_bf": xg_bf, "xgT": xgT, "xdT": xdT,
            "gidx": gidx[k], "glo": glo[k], "gr": gr[k],
            "didx": didx[k], "dlo": dlo[k], "dr": dr[k],
            "iota": iota_np, "ident": ident_np,
        }
        for name in ("w1_dg_l", "w1_dg_r", "w1_gd_l", "w1_gd_r",
                     "w2_dg_l", "w2_dg_r", "w2_gd_l", "w2_gd_r"):
            key = name.replace("_", "")[1:]          # w1_dg_l -> 1dgl
            m["w" + key] = np.asarray(inputs[name], np.float32)
        for name in ("b1_dg", "b1_gd", "b2_dg", "b2_gd"):
            m[name.replace("_", "")] = np.asarray(
                inputs[name], np.float32).reshape(F, 1)
        in_maps.append(m)
    return in_maps, (twg, twd, totg, totd)


_PROGRAM_CACHE = {}


def kernel(**inputs):
    nd, ng = ND, NG
    in_maps, (twg, twd, totg, totd) = _prepare(inputs, nd, ng)
    key = (nd, ng, tuple(twg), tuple(twd))
    if key not in _PROGRAM_CACHE:
        _PROGRAM_CACHE[key] = _build_program(nd, ng, twg, twd, totg, totd)
    nc = _PROGRAM_CACHE[key]
    res = run_bass_kernel_spmd(nc, in_maps, list(range(NCORES)))
    d2 = np.concatenate([res.results[k]["d2o"] for k in range(NCORES)], axis=0)
    g2 = np.concatenate([res.results[k]["g2o"] for k in range(NCORES)], axis=0)
    return d2, g2


# revision 18
# speedup vs baseline: 3436.0861x; 3436.0861x over previous
"""Trainium2 Bass kernel for a 2-layer bipartite GraphSAGE encoder
(disease<->gene, 25000/20000 nodes, 1M edges, all feature dims 128).

Strategy (8 NeuronCores, SPMD):
  - Shard aggregation *destinations*: core k owns genes [2500k, 2500k+2500)
    and diseases [3125k, 3125k+3125). Host sorts edges by dst (for gene
    aggregation) and by src (for disease aggregation); each core processes
    exactly the edges that target its node range, so no all-reduce of
    segment sums is needed -- only an AllGather of layer-1 node features.
  - Segment-mean on device: per 128-edge tile, dma_gather the source rows
    (bf16, edges on partitions), build onehot[e,b] = (iota==lo_e) * r_e on
    the vector engine (r_e = 1/max(deg,1) folds the mean divide), and
    accumulate meanT[f,b] += msg[e,f]^T @ onehot on the tensor engine (PSUM,
    one 128-bucket window per accumulation group).
  - Dense stages run in transposed layout: hT = W_l^T meanT + W_r^T xT + b,
    f32. Layer-1 shard results are PE-transposed to row-major bf16 tables
    and AllGathered for layer-2 gathers; final outputs are PE-transposed to
    row-major f32 shards and concatenated on host.
"""
import sys

if '/opt/trn_rl_repo' not in sys.path:
    sys.path.insert(0, '/opt/trn_rl_repo')

import numpy as np

import bass_rust
import concourse.bacc as bacc
import concourse.mybir as mybir
import concourse.tile as tile
from concourse.bass_utils import run_bass_kernel_spmd
from concourse.vector_clock import ScopedClock, VectorClock

F32 = mybir.dt.float32
BF16 = mybir.dt.bfloat16
I16 = mybir.dt.int16
NP_BF16 = mybir.dt.np(BF16)

ND, NG, E = 25000, 20000, 1_000_000
F = 128
NCORES = 8


class _TileContextSplitDrain(tile.TileContext):
    """Work around a walrus limit on sync-wait commands per Drain: emit one
    drain per (proc, tick) of the global clock, each with <=1 sem wait."""

    def _drain_and_barrier(self, tick_clock, wait_clock):
        gvc = tick_clock.global_clock
        n = len(gvc)
        for proc in range(n):
            t = gvc[proc]
            if t > 0:
                vec = [0] * n
                vec[proc] = t
                d = self.nc.sync.drain()
                wait_clock.add_sem_waits(d.ins, ScopedClock({None: VectorClock(vec)}))
        self.nc.all_engine_barrier()
        assert self.sems is not None
        popped = self.nc._tile_sem_poison_stack.pop()
        assert popped is self._sem_poison
        self.nc.clear_and_free_semaphores(list(self.sems.allocated().values()))
        self.nc.all_engine_barrier()


def _pass_meta(sortk, other, r_node, n_own, nw, ncores):
    """Per-core gather/one-hot metadata for one aggregation direction.

    sortk: [E] destination node id of each edge (aggregation key)
    other: [E] node id each edge gathers its message from
    r_node: [Ndst] f32 1/max(degree,1)
    n_own:  destination nodes per core (contiguous ranges, in core order)
    nw:     windows (of 128 destination buckets) per core

    Returns (idx16 [C,128,8*TOT] i16, lo [C,128,TOT] f32, rv [C,128,TOT] f32,
             tw list[nw], TOT).
    """
    order = np.argsort(sortk, kind="stable")
    ks = np.asarray(sortk)[order].astype(np.int64)
    os_ = np.asarray(other)[order].astype(np.int64)
    bounds = np.searchsorted(ks, np.arange(ncores + 1) * n_own)

    percore = []
    cnt_mat = np.zeros((ncores, nw), np.int64)
    for k in range(ncores):
        s, e = int(bounds[k]), int(bounds[k + 1])
        loc = ks[s:e] - k * n_own
        w = loc >> 7
        cnt_mat[k] = np.bincount(w, minlength=nw)
        percore.append((s, e, loc, w))
    tw = np.maximum(1, -(-cnt_mat // 128)).max(axis=0)          # [nw]
    To = np.concatenate([[0], np.cumsum(tw)]).astype(np.int64)  # [nw+1]
    tot = int(To[-1])

    idx16 = np.zeros((ncores, 16, 8 * tot), np.int16)
    ohs = np.zeros((ncores, 128, tot, 128), NP_BF16)
    for k in range(ncores):
        s, e, loc, w = percore[k]
        n = e - s
        if n == 0:
            continue
        offs = np.concatenate([[0], np.cumsum(cnt_mat[k])])[:-1]
        j = np.arange(n) - offs[w]          # slot within window
        tcol = To[w] + (j >> 7)             # tile column
        lane = j & 127
        ohs[k, lane, tcol, loc - (w << 7)] = r_node[ks[s:e]].astype(NP_BF16)
        icol = 8 * To[w] + (j >> 4)
        idx16[k, j & 15, icol] = os_[s:e].astype(np.int16)
    idx_full = np.tile(idx16, (1, 8, 1))    # replicate across 8x16 partitions
    return idx_full, ohs.reshape(ncores, 128, tot * 128), [int(x) for x in tw], tot


def _build_program(nd, ng, twg, twd, totg, totd, repeat=1):
    d_sh, g_sh = nd // NCORES, ng // NCORES
    gw, dw = len(twg), len(twd)
    g_pad, d_pad = gw * 128, dw * 128

    nc = bacc.Bacc("TRN2", num_devices=NCORES, num_swdge_queues=4)

    def inp(name, shape, dt):
        return nc.dram_tensor(name, shape, dt, kind="ExternalInput")

    xd_bf = inp("xd_bf", [nd, F], BF16)
    xg_bf = inp("xg_bf", [ng, F], BF16)
    xgT = inp("xgT", [F, g_pad], F32)
    xdT = inp("xdT", [F, d_pad], F32)
    gidx = inp("gidx", [128, 8 * totg], I16)
    ohsg = inp("ohsg", [128, totg * 128], BF16)
    didx = inp("didx", [128, 8 * totd], I16)
    ohsd = inp("ohsd", [128, totd * 128], BF16)
    wts = {w: inp(w, [F, F], F32)
           for w in ("w1dgl", "w1dgr", "w1gdl", "w1gdr",
                     "w2dgl", "w2dgr", "w2gdl", "w2gdr")}
    bss = {b: inp(b, [F, 1], F32) for b in ("b1dg", "b1gd", "b2dg", "b2gd")}
    ident_in = inp("ident", [128, 128], F32)
    g2o = nc.dram_tensor("g2o", [g_sh, F], F32, kind="ExternalOutput")
    d2o = nc.dram_tensor("d2o", [d_sh, F], F32, kind="ExternalOutput")

    h1g_full = nc.dram_tensor("h1g_full", [ng, F], BF16, addr_space="Shared")
    h1d_full = nc.dram_tensor("h1d_full", [nd, F], BF16, addr_space="Shared")

    with _TileContextSplitDrain(nc) as tc:
        with (
            tc.tile_pool(name="const", bufs=1) as cpool,
            tc.tile_pool(name="meanT", bufs=1) as mpool,
            tc.tile_pool(name="stage", bufs=4) as spool,
            tc.tile_pool(name="oh", bufs=2) as ohpool,
            tc.tile_pool(name="tmp", bufs=3) as tpool,
            tc.tile_pool(name="agg_ps", bufs=3, space="PSUM") as agg_ps,
            tc.tile_pool(name="dense_ps", bufs=2, space="PSUM") as dense_ps,
            tc.tile_pool(name="tp_ps", bufs=2, space="PSUM") as tp_ps,
            tc.tile_pool(name="dram", bufs=1, space="DRAM") as dram,
        ):
            def load_const(src, shape, dt):
                t = cpool.tile(shape, dt, tag=src.name)
                nc.sync.dma_start(t[:], src[:])
                return t

            ident_sb = load_const(ident_in, [128, 128], F32)
            w_sb = {k: load_const(v, [F, F], F32) for k, v in wts.items()}
            b_sb = {k: load_const(v, [F, 1], F32) for k, v in bss.items()}
            xgT_sb = load_const(xgT, [F, g_pad], F32)
            xdT_sb = load_const(xdT, [F, d_pad], F32)
            gidx_sb = load_const(gidx, [128, 8 * totg], I16)
            didx_sb = load_const(didx, [128, 8 * totd], I16)

            h1gT = cpool.tile([128, g_pad], F32, tag="h1gT")
            h1dT = cpool.tile([128, d_pad], F32, tag="h1dT")

            h1g_rows = dram.tile([g_sh, F], BF16)
            h1d_rows = dram.tile([d_sh, F], BF16)

            qctr = [0]

            def agg(table_ap, idx_sb, ohs_ap, tw, meanT, dep=None):
                # one window per dma_gather, rotating over the 4 SWDGE queues;
                # one-hot tiles are host-precomputed and streamed via HWDGE
                to = 0
                for w, tww in enumerate(tw):
                    stage = spool.tile([128, tww * F], BF16, tag="stage")
                    g = nc.gpsimd.dma_gather(
                        out_ap=stage[:].rearrange("p (t f) -> p t f", f=F),
                        in_ap=table_ap,
                        idxs_ap=idx_sb[:, 8 * to:8 * (to + tww)],
                        num_idxs=tww * 128,
                        num_idxs_reg=tww * 128,
                        elem_size=F,
                        single_packet=False,
                        queue_num=qctr[0] % 4,
                    )
                    qctr[0] += 1
                    if dep is not None:
                        bass_rust.add_dep_helper(
                            g.ins, dep.ins, reason="gather reads shared CC output")
                    ohst = ohpool.tile([128, tww * F], BF16, tag="ohst")
                    nc.sync.dma_start(ohst[:], ohs_ap[:, to * F:(to + tww) * F])
                    ps = agg_ps.tile([128, 128], F32, tag="agg")
                    for t in range(tww):
                        nc.tensor.matmul(
                            ps[:], lhsT=stage[:, t * F:(t + 1) * F],
                            rhs=ohst[:, t * F:(t + 1) * F],
                            start=(t == 0), stop=(t == tww - 1),
                        )
                    nc.vector.tensor_copy(meanT[:, w * 128:(w + 1) * 128], ps[:])
                    to += tww

            def dense(meanT, selfT, wl, wr, bias, outT, npad):
                for c0 in range(0, npad, 512):
                    n = min(512, npad - c0)
                    ps = dense_ps.tile([128, 512], F32, tag="dense")
                    nc.tensor.matmul(ps[:, :n], lhsT=wl[:], rhs=meanT[:, c0:c0 + n],
                                     start=True, stop=False)
                    nc.tensor.matmul(ps[:, :n], lhsT=wr[:], rhs=selfT[:, c0:c0 + n],
                                     start=False, stop=True)
                    nc.vector.tensor_scalar(
                        outT[:, c0:c0 + n], ps[:, :n], bias[:], None,
                        mybir.AluOpType.add,
                    )

            def emit_rows(srcT, n_real, rows_dram, dt):
                for b in range(-(-n_real // 128)):
                    rows = min(128, n_real - b * 128)
                    tp = tp_ps.tile([128, 128], F32, tag="tp")
                    nc.tensor.transpose(tp[:], srcT[:, b * 128:(b + 1) * 128],
                                        ident_sb[:])
                    sb = tpool.tile([128, 128], dt, tag=f"emit_{dt}")
                    nc.vector.tensor_copy(sb[:], tp[:])
                    nc.sync.dma_start(rows_dram[b * 128:b * 128 + rows, :],
                                      sb[:rows, :])

            for _rep in range(repeat):
                _run_pipeline(nc, mpool, agg, dense, emit_rows, w_sb, b_sb,
                              xd_bf, xg_bf, xgT_sb, xdT_sb,
                              gidx_sb, ohsg, didx_sb, ohsd, h1gT, h1dT,
                              h1g_rows, h1d_rows, h1g_full, h1d_full,
                              twg, twd, g_pad, d_pad, g_sh, d_sh, g2o, d2o)

    nc.compile()
    return nc


def _run_pipeline(nc, mpool, agg, dense, emit_rows, w_sb, b_sb,
                  xd_bf, xg_bf, xgT_sb, xdT_sb,
                  gidx_sb, ohsg, didx_sb, ohsd, h1gT, h1dT,
                  h1g_rows, h1d_rows, h1g_full, h1d_full,
                  twg, twd, g_pad, d_pad, g_sh, d_sh, g2o, d2o):
            # ---- layer 1, gene side; its AllGather overlaps the disease side
            meanTg = mpool.tile([128, g_pad], F32, tag="meanTg")
            agg(xd_bf[:], gidx_sb, ohsg[:], twg, meanTg)
            dense(meanTg, xgT_sb, w_sb["w1dgl"], w_sb["w1dgr"], b_sb["b1dg"],
                  h1gT, g_pad)
            emit_rows(h1gT, g_sh, h1g_rows, BF16)
            cc_g = nc.gpsimd.collective_compute(
                "AllGather", mybir.AluOpType.bypass,
                replica_groups=[list(range(NCORES))],
                ins=[h1g_rows[:]], outs=[h1g_full[:]],
            )

            # ---- layer 1, disease side; its AllGather overlaps L2 disease
            meanTd = mpool.tile([128, d_pad], F32, tag="meanTd")
            agg(xg_bf[:], didx_sb, ohsd[:], twd, meanTd)
            dense(meanTd, xdT_sb, w_sb["w1gdl"], w_sb["w1gdr"], b_sb["b1gd"],
                  h1dT, d_pad)
            emit_rows(h1dT, d_sh, h1d_rows, BF16)
            cc_d = nc.gpsimd.collective_compute(
                "AllGather", mybir.AluOpType.bypass,
                replica_groups=[list(range(NCORES))],
                ins=[h1d_rows[:]], outs=[h1d_full[:]],
            )

            # ---- layer 2, disease side first (needs only h1g_full)
            meanTd2 = mpool.tile([128, d_pad], F32, tag="meanTd")
            agg(h1g_full[:], didx_sb, ohsd[:], twd, meanTd2, dep=cc_g)
            d2T = mpool.tile([128, d_pad], F32, tag="meanTg")
            dense(meanTd2, h1dT, w_sb["w2gdl"], w_sb["w2gdr"], b_sb["b2gd"],
                  d2T, d_pad)
            emit_rows(d2T, d_sh, d2o[:], F32)

            # ---- layer 2, gene side
            meanTg2 = mpool.tile([128, g_pad], F32, tag="meanTg")
            agg(h1d_full[:], gidx_sb, ohsg[:], twg, meanTg2, dep=cc_d)
            g2T = mpool.tile([128, g_pad], F32, tag="meanTd")
            dense(meanTg2, h1gT, w_sb["w2dgl"], w_sb["w2dgr"], b_sb["b2dg"],
                  g2T, g_pad)
            emit_rows(g2T, g_sh, g2o[:], F32)


def _prepare(inputs, nd, ng):
    """Host-side sharding + metadata. Returns (in_maps, static) where static
    carries the per-pass tile counts baked into the program."""
    d_sh, g_sh = nd // NCORES, ng // NCORES
    gw = -(-g_sh // 128)
    dw = -(-d_sh // 128)
    g_pad, d_pad = gw * 128, dw * 128

    x_d = np.asarray(inputs["x_disease"], np.float32)
    x_g = np.asarray(inputs["x_gene"], np.float32)
    src = np.asarray(inputs["src"], np.int64)
    dst = np.asarray(inputs["dst"], np.int64)

    cnt_g = np.bincount(dst, minlength=ng).astype(np.float32)
    cnt_d = np.bincount(src, minlength=nd).astype(np.float32)
    r_g = (1.0 / np.maximum(cnt_g, 1.0)).astype(np.float32)
    r_d = (1.0 / np.maximum(cnt_d, 1.0)).astype(np.float32)

    gidx, ohsg, twg, totg = _pass_meta(dst, src, r_g, g_sh, gw, NCORES)
    didx, ohsd, twd, totd = _pass_meta(src, dst, r_d, d_sh, dw, NCORES)

    ident_np = np.eye(128, dtype=np.float32)
    xd_bf = x_d.astype(NP_BF16)
    xg_bf = x_g.astype(NP_BF16)

    in_maps = []
    for k in range(NCORES):
        xgT = np.zeros((F, g_pad), np.float32)
        xgT[:, :g_sh] = x_g[k * g_sh:(k + 1) * g_sh].T
        xdT = np.zeros((F, d_pad), np.float32)
        xdT[:, :d_sh] = x_d[k * d_sh:(k + 1) * d_sh].T
        m = {
            "xd_bf": xd_bf, "xg_bf": xg_bf, "xgT": xgT, "xdT": xdT,
            "gidx": gidx[k], "ohsg": ohsg[k],
            "didx": didx[k], "ohsd": ohsd[k],
            "ident": ident_np,
        }
        for name in ("w1_dg_l", "w1_dg_r", "w1_gd_l", "w1_gd_r",
                     "w2_dg_l", "w2_dg_r", "w2_gd_l", "w2_gd_r"):
            key = name.replace("_", "")[1:]          # w1_dg_l -> 1dgl
            m["w" + key] = np.asarray(inputs[name], np.float32)
        for name in ("b1_dg", "b1_gd", "b2_dg", "b2_gd"):
            m[name.replace("_", "")] = np.asarray(
                inputs[name], np.float32).reshape(F, 1)
        in_maps.append(m)
    return in_maps, (twg, twd, totg, totd)


_PROGRAM_CACHE = {}


def kernel(**inputs):
    nd, ng = ND, NG
    in_maps, (twg, twd, totg, totd) = _prepare(inputs, nd, ng)
    key = (nd, ng, tuple(twg), tuple(twd))
    if key not in _PROGRAM_CACHE:
        _PROGRAM_CACHE[key] = _build_program(nd, ng, twg, twd, totg, totd)
    nc = _PROGRAM_CACHE[key]
    res = run_bass_kernel_spmd(nc, in_maps, list(range(NCORES)))
    d2 = np.concatenate([res.results[k]["d2o"] for k in range(NCORES)], axis=0)
    g2 = np.concatenate([res.results[k]["g2o"] for k in range(NCORES)], axis=0)
    return d2, g2


# revision 19
# speedup vs baseline: 6121.7626x; 1.7816x over previous
"""Trainium2 Bass kernel for a 2-layer bipartite GraphSAGE encoder
(disease<->gene, 25000/20000 nodes, 1M edges, all feature dims 128).

Strategy (8 NeuronCores, SPMD):
  - Shard aggregation *destinations*: core k owns genes [2500k, 2500k+2500)
    and diseases [3125k, 3125k+3125). Host sorts edges by dst (for gene
    aggregation) and by src (for disease aggregation); each core processes
    exactly the edges that target its node range, so no all-reduce of
    segment sums is needed -- only an AllGather of layer-1 node features.
  - Segment-mean on device: per 128-edge tile, dma_gather the source rows
    (bf16, edges on partitions), build onehot[e,b] = (iota==lo_e) * r_e on
    the vector engine (r_e = 1/max(deg,1) folds the mean divide), and
    accumulate meanT[f,b] += msg[e,f]^T @ onehot on the tensor engine (PSUM,
    one 128-bucket window per accumulation group).
  - Dense stages run in transposed layout: hT = W_l^T meanT + W_r^T xT + b,
    f32. Layer-1 shard results are PE-transposed to row-major bf16 tables
    and AllGathered for layer-2 gathers; final outputs are PE-transposed to
    row-major f32 shards and concatenated on host.
"""
import sys

if '/opt/trn_rl_repo' not in sys.path:
    sys.path.insert(0, '/opt/trn_rl_repo')

import numpy as np

import bass_rust
import concourse.bacc as bacc
import concourse.mybir as mybir
import concourse.tile as tile
from concourse.bass_utils import run_bass_kernel_spmd
from concourse.vector_clock import ScopedClock, VectorClock

F32 = mybir.dt.float32
BF16 = mybir.dt.bfloat16
I16 = mybir.dt.int16
NP_BF16 = mybir.dt.np(BF16)

ND, NG, E = 25000, 20000, 1_000_000
F = 128
NCORES = 8


class _TileContextSplitDrain(tile.TileContext):
    """Work around a walrus limit on sync-wait commands per Drain: emit one
    drain per (proc, tick) of the global clock, each with <=1 sem wait."""

    def _drain_and_barrier(self, tick_clock, wait_clock):
        gvc = tick_clock.global_clock
        n = len(gvc)
        for proc in range(n):
            t = gvc[proc]
            if t > 0:
                vec = [0] * n
                vec[proc] = t
                d = self.nc.sync.drain()
                wait_clock.add_sem_waits(d.ins, ScopedClock({None: VectorClock(vec)}))
        self.nc.all_engine_barrier()
        assert self.sems is not None
        popped = self.nc._tile_sem_poison_stack.pop()
        assert popped is self._sem_poison
        self.nc.clear_and_free_semaphores(list(self.sems.allocated().values()))
        self.nc.all_engine_barrier()


def _pass_meta(sortk, other, r_node, n_own, nw, ncores):
    """Per-core gather/one-hot metadata for one aggregation direction.

    sortk: [E] destination node id of each edge (aggregation key)
    other: [E] node id each edge gathers its message from
    r_node: [Ndst] f32 1/max(degree,1)
    n_own:  destination nodes per core (contiguous ranges, in core order)
    nw:     windows (of 128 destination buckets) per core

    Returns (idx16 [C,128,8*TOT] i16, lo [C,128,TOT] f32, rv [C,128,TOT] f32,
             tw list[nw], TOT).
    """
    order = np.argsort(sortk, kind="stable")
    ks = np.asarray(sortk)[order].astype(np.int64)
    os_ = np.asarray(other)[order].astype(np.int64)
    bounds = np.searchsorted(ks, np.arange(ncores + 1) * n_own)

    percore = []
    cnt_mat = np.zeros((ncores, nw), np.int64)
    for k in range(ncores):
        s, e = int(bounds[k]), int(bounds[k + 1])
        loc = ks[s:e] - k * n_own
        w = loc >> 7
        cnt_mat[k] = np.bincount(w, minlength=nw)
        percore.append((s, e, loc, w))
    tw = np.maximum(1, -(-cnt_mat // 128)).max(axis=0)          # [nw]
    To = np.concatenate([[0], np.cumsum(tw)]).astype(np.int64)  # [nw+1]
    tot = int(To[-1])

    idx16 = np.zeros((ncores, 16, 8 * tot), np.int16)
    ohs = np.zeros((ncores, 128, tot, 128), NP_BF16)
    for k in range(ncores):
        s, e, loc, w = percore[k]
        n = e - s
        if n == 0:
            continue
        offs = np.concatenate([[0], np.cumsum(cnt_mat[k])])[:-1]
        j = np.arange(n) - offs[w]          # slot within window
        tcol = To[w] + (j >> 7)             # tile column
        lane = j & 127
        ohs[k, lane, tcol, loc - (w << 7)] = r_node[ks[s:e]].astype(NP_BF16)
        icol = 8 * To[w] + (j >> 4)
        idx16[k, j & 15, icol] = os_[s:e].astype(np.int16)
    idx_full = np.tile(idx16, (1, 8, 1))    # replicate across 8x16 partitions
    return idx_full, ohs.reshape(ncores, 128, tot * 128), [int(x) for x in tw], tot


def _build_program(nd, ng, twg, twd, totg, totd, repeat=1):
    d_sh, g_sh = nd // NCORES, ng // NCORES
    gw, dw = len(twg), len(twd)
    g_pad, d_pad = gw * 128, dw * 128

    nc = bacc.Bacc("TRN2", num_devices=NCORES, num_swdge_queues=4)

    def inp(name, shape, dt):
        return nc.dram_tensor(name, shape, dt, kind="ExternalInput")

    xd_bf = inp("xd_bf", [nd, F], BF16)
    xg_bf = inp("xg_bf", [ng, F], BF16)
    xgT = inp("xgT", [F, g_pad], F32)
    xdT = inp("xdT", [F, d_pad], F32)
    gidx = inp("gidx", [128, 8 * totg], I16)
    ohsg = inp("ohsg", [128, totg * 128], BF16)
    didx = inp("didx", [128, 8 * totd], I16)
    ohsd = inp("ohsd", [128, totd * 128], BF16)
    wts = {w: inp(w, [F, F], F32)
           for w in ("w1dgl", "w1dgr", "w1gdl", "w1gdr",
                     "w2dgl", "w2dgr", "w2gdl", "w2gdr")}
    bss = {b: inp(b, [F, 1], F32) for b in ("b1dg", "b1gd", "b2dg", "b2gd")}
    ident_in = inp("ident", [128, 128], F32)
    g2o = nc.dram_tensor("g2o", [g_sh, F], F32, kind="ExternalOutput")
    d2o = nc.dram_tensor("d2o", [d_sh, F], F32, kind="ExternalOutput")

    h1g_full = nc.dram_tensor("h1g_full", [ng, F], BF16, addr_space="Shared")
    h1d_full = nc.dram_tensor("h1d_full", [nd, F], BF16, addr_space="Shared")

    with _TileContextSplitDrain(nc) as tc:
        with (
            tc.tile_pool(name="const", bufs=1) as cpool,
            tc.tile_pool(name="meanT", bufs=1) as mpool,
            tc.tile_pool(name="stage", bufs=5) as spool,
            tc.tile_pool(name="oh", bufs=2) as ohpool,
            tc.tile_pool(name="tmp", bufs=3) as tpool,
            tc.tile_pool(name="agg_ps", bufs=3, space="PSUM") as agg_ps,
            tc.tile_pool(name="dense_ps", bufs=2, space="PSUM") as dense_ps,
            tc.tile_pool(name="tp_ps", bufs=2, space="PSUM") as tp_ps,
            tc.tile_pool(name="dram", bufs=1, space="DRAM") as dram,
        ):
            def load_const(src, shape, dt):
                t = cpool.tile(shape, dt, tag=src.name)
                nc.sync.dma_start(t[:], src[:])
                return t

            ident_sb = load_const(ident_in, [128, 128], F32)
            w_sb = {k: load_const(v, [F, F], F32) for k, v in wts.items()}
            b_sb = {k: load_const(v, [F, 1], F32) for k, v in bss.items()}
            xgT_sb = load_const(xgT, [F, g_pad], F32)
            xdT_sb = load_const(xdT, [F, d_pad], F32)
            gidx_sb = load_const(gidx, [128, 8 * totg], I16)
            didx_sb = load_const(didx, [128, 8 * totd], I16)

            h1gT = cpool.tile([128, g_pad], F32, tag="h1gT")
            h1dT = cpool.tile([128, d_pad], F32, tag="h1dT")

            h1g_rows = dram.tile([g_sh, F], BF16)
            h1d_rows = dram.tile([d_sh, F], BF16)

            qctr = [0]

            def agg(table_ap, idx_sb, ohs_ap, tw, meanT, dep=None):
                # one window per dma_gather, rotating over the 4 SWDGE queues;
                # one-hot tiles are host-precomputed and streamed via HWDGE
                to = 0
                for w, tww in enumerate(tw):
                    stage = spool.tile([128, tww * F], BF16, tag="stage")
                    g = nc.gpsimd.dma_gather(
                        out_ap=stage[:].rearrange("p (t f) -> p t f", f=F),
                        in_ap=table_ap,
                        idxs_ap=idx_sb[:, 8 * to:8 * (to + tww)],
                        num_idxs=tww * 128,
                        num_idxs_reg=tww * 128,
                        elem_size=F,
                        single_packet=False,
                        queue_num=qctr[0] % 4,
                    )
                    qctr[0] += 1
                    if dep is not None:
                        bass_rust.add_dep_helper(
                            g.ins, dep.ins, reason="gather reads shared CC output")
                    ohst = ohpool.tile([128, tww * F], BF16, tag="ohst")
                    nc.sync.dma_start(ohst[:], ohs_ap[:, to * F:(to + tww) * F])
                    ps = agg_ps.tile([128, 128], F32, tag="agg")
                    for t in range(tww):
                        nc.tensor.matmul(
                            ps[:], lhsT=stage[:, t * F:(t + 1) * F],
                            rhs=ohst[:, t * F:(t + 1) * F],
                            start=(t == 0), stop=(t == tww - 1),
                        )
                    nc.vector.tensor_copy(meanT[:, w * 128:(w + 1) * 128], ps[:])
                    to += tww

            def dense(meanT, selfT, wl, wr, bias, outT, npad):
                for c0 in range(0, npad, 512):
                    n = min(512, npad - c0)
                    ps = dense_ps.tile([128, 512], F32, tag="dense")
                    nc.tensor.matmul(ps[:, :n], lhsT=wl[:], rhs=meanT[:, c0:c0 + n],
                                     start=True, stop=False)
                    nc.tensor.matmul(ps[:, :n], lhsT=wr[:], rhs=selfT[:, c0:c0 + n],
                                     start=False, stop=True)
                    nc.vector.tensor_scalar(
                        outT[:, c0:c0 + n], ps[:, :n], bias[:], None,
                        mybir.AluOpType.add,
                    )

            def emit_rows(srcT, n_real, rows_dram, dt):
                for b in range(-(-n_real // 128)):
                    rows = min(128, n_real - b * 128)
                    tp = tp_ps.tile([128, 128], F32, tag="tp")
                    nc.tensor.transpose(tp[:], srcT[:, b * 128:(b + 1) * 128],
                                        ident_sb[:])
                    sb = tpool.tile([128, 128], dt, tag=f"emit_{dt}")
                    nc.vector.tensor_copy(sb[:], tp[:])
                    nc.sync.dma_start(rows_dram[b * 128:b * 128 + rows, :],
                                      sb[:rows, :])

            for _rep in range(repeat):
                _run_pipeline(nc, mpool, agg, dense, emit_rows, w_sb, b_sb,
                              xd_bf, xg_bf, xgT_sb, xdT_sb,
                              gidx_sb, ohsg, didx_sb, ohsd, h1gT, h1dT,
                              h1g_rows, h1d_rows, h1g_full, h1d_full,
                              twg, twd, g_pad, d_pad, g_sh, d_sh, g2o, d2o)

    nc.compile()
    return nc


def _run_pipeline(nc, mpool, agg, dense, emit_rows, w_sb, b_sb,
                  xd_bf, xg_bf, xgT_sb, xdT_sb,
                  gidx_sb, ohsg, didx_sb, ohsd, h1gT, h1dT,
                  h1g_rows, h1d_rows, h1g_full, h1d_full,
                  twg, twd, g_pad, d_pad, g_sh, d_sh, g2o, d2o):
            # ---- layer 1, gene side; its AllGather overlaps the disease side
            meanTg = mpool.tile([128, g_pad], F32, tag="meanTg")
            agg(xd_bf[:], gidx_sb, ohsg[:], twg, meanTg)
            dense(meanTg, xgT_sb, w_sb["w1dgl"], w_sb["w1dgr"], b_sb["b1dg"],
                  h1gT, g_pad)
            emit_rows(h1gT, g_sh, h1g_rows, BF16)
            cc_g = nc.gpsimd.collective_compute(
                "AllGather", mybir.AluOpType.bypass,
                replica_groups=[list(range(NCORES))],
                ins=[h1g_rows[:]], outs=[h1g_full[:]],
            )

            # ---- layer 1, disease side; its AllGather overlaps L2 disease
            meanTd = mpool.tile([128, d_pad], F32, tag="meanTd")
            agg(xg_bf[:], didx_sb, ohsd[:], twd, meanTd)
            dense(meanTd, xdT_sb, w_sb["w1gdl"], w_sb["w1gdr"], b_sb["b1gd"],
                  h1dT, d_pad)
            emit_rows(h1dT, d_sh, h1d_rows, BF16)
            cc_d = nc.gpsimd.collective_compute(
                "AllGather", mybir.AluOpType.bypass,
                replica_groups=[list(range(NCORES))],
                ins=[h1d_rows[:]], outs=[h1d_full[:]],
            )

            # ---- layer 2, disease side first (needs only h1g_full)
            meanTd2 = mpool.tile([128, d_pad], F32, tag="meanTd")
            agg(h1g_full[:], didx_sb, ohsd[:], twd, meanTd2, dep=cc_g)
            d2T = mpool.tile([128, d_pad], F32, tag="meanTg")
            dense(meanTd2, h1dT, w_sb["w2gdl"], w_sb["w2gdr"], b_sb["b2gd"],
                  d2T, d_pad)
            emit_rows(d2T, d_sh, d2o[:], F32)

            # ---- layer 2, gene side
            meanTg2 = mpool.tile([128, g_pad], F32, tag="meanTg")
            agg(h1d_full[:], gidx_sb, ohsg[:], twg, meanTg2, dep=cc_d)
            g2T = mpool.tile([128, g_pad], F32, tag="meanTd")
            dense(meanTg2, h1gT, w_sb["w2dgl"], w_sb["w2dgr"], b_sb["b2dg"],
                  g2T, g_pad)
            emit_rows(g2T, g_sh, g2o[:], F32)


def _prepare(inputs, nd, ng):
    """Host-side sharding + metadata. Returns (in_maps, static) where static
    carries the per-pass tile counts baked into the program."""
    d_sh, g_sh = nd // NCORES, ng // NCORES
    gw = -(-g_sh // 128)
    dw = -(-d_sh // 128)
    g_pad, d_pad = gw * 128, dw * 128

    x_d = np.asarray(inputs["x_disease"], np.float32)
    x_g = np.asarray(inputs["x_gene"], np.float32)
    src = np.asarray(inputs["src"], np.int64)
    dst = np.asarray(inputs["dst"], np.int64)

    cnt_g = np.bincount(dst, minlength=ng).astype(np.float32)
    cnt_d = np.bincount(src, minlength=nd).astype(np.float32)
    r_g = (1.0 / np.maximum(cnt_g, 1.0)).astype(np.float32)
    r_d = (1.0 / np.maximum(cnt_d, 1.0)).astype(np.float32)

    gidx, ohsg, twg, totg = _pass_meta(dst, src, r_g, g_sh, gw, NCORES)
    didx, ohsd, twd, totd = _pass_meta(src, dst, r_d, d_sh, dw, NCORES)

    ident_np = np.eye(128, dtype=np.float32)
    xd_bf = x_d.astype(NP_BF16)
    xg_bf = x_g.astype(NP_BF16)

    in_maps = []
    for k in range(NCORES):
        xgT = np.zeros((F, g_pad), np.float32)
        xgT[:, :g_sh] = x_g[k * g_sh:(k + 1) * g_sh].T
        xdT = np.zeros((F, d_pad), np.float32)
        xdT[:, :d_sh] = x_d[k * d_sh:(k + 1) * d_sh].T
        m = {
            "xd_bf": xd_bf, "xg_bf": xg_bf, "xgT": xgT, "xdT": xdT,
            "gidx": gidx[k], "ohsg": ohsg[k],
            "didx": didx[k], "ohsd": ohsd[k],
            "ident": ident_np,
        }
        for name in ("w1_dg_l", "w1_dg_r", "w1_gd_l", "w1_gd_r",
                     "w2_dg_l", "w2_dg_r", "w2_gd_l", "w2_gd_r"):
            key = name.replace("_", "")[1:]          # w1_dg_l -> 1dgl
            m["w" + key] = np.asarray(inputs[name], np.float32)
        for name in ("b1_dg", "b1_gd", "b2_dg", "b2_gd"):
            m[name.replace("_", "")] = np.asarray(
                inputs[name], np.float32).reshape(F, 1)
        in_maps.append(m)
    return in_maps, (twg, twd, totg, totd)


_PROGRAM_CACHE = {}


def kernel(**inputs):
    nd, ng = ND, NG
    in_maps, (twg, twd, totg, totd) = _prepare(inputs, nd, ng)
    key = (nd, ng, tuple(twg), tuple(twd))
    if key not in _PROGRAM_CACHE:
        _PROGRAM_CACHE[key] = _build_program(nd, ng, twg, twd, totg, totd)
    nc = _PROGRAM_CACHE[key]
    res = run_bass_kernel_spmd(nc, in_maps, list(range(NCORES)))
    d2 = np.concatenate([res.results[k]["d2o"] for k in range(NCORES)], axis=0)
    g2 = np.concatenate([res.results[k]["g2o"] for k in range(NCORES)], axis=0)
    return d2, g2
